# revision 11
# baseline (speedup 1.0000x reference)
"""Trainium2 Bass kernel for per-position head-attention (nn_DariushFlashAttention2).

Math (per batch b, sequence position s):
    Q = q[b,s].reshape(H=32, D=128); K, V likewise
    logits = Q @ K.T / sqrt(D)          # [32, 32] attention over HEADS
    W = softmax(logits, axis=-1)
    out[b,s] = (W @ V).reshape(H*D)

Every one of the B*S = 8192 positions is independent, so we shard positions
across the 8 NeuronCores (1024 positions each) and run one SPMD program.

Device strategy (per core):
  - Positions are packed 4-per-"group" onto the 128 SBUF partitions
    (partition = 4*32 = pos_in_group x head).
  - Host pre-transposes q,k into [d, (pos,h)] layout and pre-casts to fp16,
    so the device needs no on-chip transposes and HBM traffic halves.
  - One 1.5 MB input DMA per 64-position chunk (q,k,v interleaved in one
    DRAM tensor) from the Sync HWDGE ring.
  - QK: per position j one col-tiled matmul (tile_position=(0,32j),
    K=128(d), M=32(k-heads), N=32(q-heads)); all 16 groups of a chunk land
    in one [128,512] PSUM bank -> a single exp() per chunk on ScalarE.
  - WV: per position a (32j,32j) sub-array matmul whose stationary operand
    is that position's [g,h] exp block read in place; V is stored
    [(pos,g), d] with a ones-column per group so the same matmul emits the
    softmax denominator in its last column.
  - Per-partition reciprocal (batched over 2 groups) + normalize while
    evacuating PSUM, alternating between ScalarE and VectorE.
  - Output halves drain early via the Scalar HWDGE ring so out-DMAs never
    head-of-line-block input prefetch on the Sync queue.
"""

import numpy as np

B, S, H, D = 2, 4096, 32, 128
NCORES = 8
POS = B * S                  # 8192 positions total
PPC = POS // NCORES          # 1024 positions per core
GP = 4                       # positions per group (4*32 heads = 128 partitions)
NG = 16                      # groups per chunk
CHUNK_POS = GP * NG          # 64 positions per chunk
NCHUNK = PPC // CHUNK_POS    # 16 chunks per core
VCOL = D + 1                 # v columns per group incl. ones column
QKW = NG * D                 # 2048 cols for qt / kt
VW = NG * VCOL               # 2064 cols for v
INW = 2 * QKW + VW           # combined input width per chunk

_SCALE = float(1.0 / np.sqrt(D))

_program = None  # cached compiled Bass program


def _build_program():
    import concourse.bacc as bacc
    import concourse.mybir as mybir
    from concourse.tile import TileContext

    fp32 = mybir.dt.float32
    fp16 = mybir.dt.float16

    nc = bacc.Bacc()
    qkv = nc.dram_tensor("qkv", [NCHUNK, 128, INW], fp16, kind="ExternalInput")
    out = nc.dram_tensor("out", [NCHUNK, 128, NG * D], fp16, kind="ExternalOutput")

    with TileContext(nc) as tc:
        with (
            tc.tile_pool(name="qkv_in", bufs=4) as in_pool,
            tc.tile_pool(name="o_out", bufs=3) as o_pool,
            tc.tile_pool(name="exp", bufs=2) as exp_pool,
            tc.tile_pool(name="small", bufs=8) as small_pool,
            tc.tile_pool(name="psl", bufs=2, space="PSUM") as psl_pool,
            tc.tile_pool(name="pso", bufs=5, space="PSUM") as pso_pool,
        ):
            for n in range(NCHUNK):
                in_t = in_pool.tile([128, INW], fp16, tag="qkv")
                nc.sync.dma_start(out=in_t, in_=qkv[n])
                qt_t = in_t[:, 0:QKW]
                kt_t = in_t[:, QKW:2 * QKW]
                vp_t = in_t[:, 2 * QKW:INW]
                out_t = o_pool.tile([128, NG * D], fp16, tag="out")

                # All 16 groups' logits^T into one [128, 512] PSUM bank.
                psum_l = psl_pool.tile([128, 32 * NG], fp32, tag="psl")
                for g in range(NG):
                    for j in range(GP):
                        c = slice(g * D + 32 * j, g * D + 32 * j + 32)
                        nc.tensor.matmul(
                            psum_l[32 * j:32 * j + 32, 32 * g:32 * g + 32],
                            kt_t[:, c],
                            qt_t[:, c],
                            start=True, stop=True,
                            tile_position=(0, 32 * j),
                        )
                exp_sb = exp_pool.tile([128, 32 * NG], fp16, tag="exp_sb")
                nc.scalar.activation(
                    exp_sb, psum_l, mybir.ActivationFunctionType.Exp,
                    scale=_SCALE,
                )

                for p2 in range(NG // 2):        # pairs of groups
                    psum_o = pso_pool.tile([128, 2 * VCOL], fp32, tag="pso")
                    for u in range(2):
                        g = 2 * p2 + u
                        for j in range(GP):
                            r = slice(32 * j, 32 * j + 32)
                            nc.tensor.matmul(
                                psum_o[r, u * VCOL:(u + 1) * VCOL],
                                exp_sb[r, 32 * g:32 * g + 32],
                                vp_t[r, g * VCOL:(g + 1) * VCOL],
                                start=True, stop=True,
                                tile_position=(32 * j, 32 * j),
                            )
                    recip = small_pool.tile([128, 2], fp32, tag="recip")
                    zcols = psum_o.rearrange("p (u c) -> p u c", c=VCOL)[:, :, D]
                    nc.vector.reciprocal(recip, zcols)
                    for u in range(2):
                        g = 2 * p2 + u
                        src = psum_o[:, u * VCOL:u * VCOL + D]
                        dst = out_t[:, g * D:(g + 1) * D]
                        if g % 2 == 0:
                            nc.scalar.activation(
                                dst, src, mybir.ActivationFunctionType.Copy,
                                scale=recip[:, u:u + 1],
                            )
                        else:
                            nc.vector.tensor_scalar_mul(dst, src, recip[:, u:u + 1])

                    # Drain finished halves early from the Scalar HWDGE ring.
                    if p2 == 3:
                        nc.scalar.dma_start(
                            out=out[n, :, :NG * D // 2], in_=out_t[:, :NG * D // 2])
                    elif p2 == 7:
                        nc.scalar.dma_start(
                            out=out[n, :, NG * D // 2:], in_=out_t[:, NG * D // 2:])

    nc.compile()
    return nc


def _host_pack(q, k, v):
    """Build per-core device input arrays from full fp32 inputs."""
    qf = np.ascontiguousarray(q, dtype=np.float32).reshape(POS, H, D)
    kf = np.ascontiguousarray(k, dtype=np.float32).reshape(POS, H, D)
    vf = np.ascontiguousarray(v, dtype=np.float32).reshape(POS, H, D)

    nchunk_tot = POS // CHUNK_POS
    # q,k: [chunk, group, i, h, d] -> [chunk, d, (group, i, h)]
    def to_qt(x):
        x = x.reshape(nchunk_tot, NG, GP, H, D)
        x = x.transpose(0, 4, 1, 2, 3)
        return x.reshape(nchunk_tot, D, NG * GP * H)

    # v: [chunk, group, i, gh, d] -> [chunk, (i,gh), (group, d|1)]
    vv = vf.reshape(nchunk_tot, NG, GP, H, D).transpose(0, 2, 3, 1, 4)
    vp_all = np.ones((nchunk_tot, GP, H, NG, VCOL), dtype=np.float32)
    vp_all[..., :D] = vv
    vp_all = vp_all.reshape(nchunk_tot, GP * H, NG * VCOL)

    comb = np.empty((nchunk_tot, 128, INW), dtype=np.float16)
    comb[:, :, 0:QKW] = to_qt(qf)
    comb[:, :, QKW:2 * QKW] = to_qt(kf)
    comb[:, :, 2 * QKW:INW] = vp_all

    in_maps = []
    for c in range(NCORES):
        sl = slice(c * NCHUNK, (c + 1) * NCHUNK)
        in_maps.append({"qkv": np.ascontiguousarray(comb[sl])})
    return in_maps


def _host_unpack(outs):
    """Per-core [NCHUNK, 128, NG*D] fp16 -> full [B, S, H*D] fp32."""
    full = np.concatenate(outs, axis=0).astype(np.float32)
    nchunk_tot = POS // CHUNK_POS
    full = full.reshape(nchunk_tot, GP, H, NG, D)   # [chunk, i, h, g, d]
    full = full.transpose(0, 3, 1, 2, 4)            # [chunk, g, i, h, d]
    return np.ascontiguousarray(full.reshape(B, S, H * D))


def kernel(q, k, v, _trace=False):
    global _program
    from concourse.bass_utils import run_bass_kernel_spmd

    if _program is None:
        _program = _build_program()

    in_maps = _host_pack(q, k, v)
    res = run_bass_kernel_spmd(_program, in_maps, list(range(NCORES)), trace=_trace)
    outs = [res.results[c]["out"] for c in range(NCORES)]
    result = _host_unpack(outs)
    if _trace:
        return result, res
    return result


# revision 12
# speedup vs baseline: 1.0212x; 1.0212x over previous
"""Trainium2 Bass kernel for per-position head-attention (nn_DariushFlashAttention2).

Math (per batch b, sequence position s):
    Q = q[b,s].reshape(H=32, D=128); K, V likewise
    logits = Q @ K.T / sqrt(D)          # [32, 32] attention over HEADS
    W = softmax(logits, axis=-1)
    out[b,s] = (W @ V).reshape(H*D)

Every one of the B*S = 8192 positions is independent, so we shard positions
across the 8 NeuronCores (1024 positions each) and run one SPMD program.

Device strategy (per core):
  - Positions are packed 4-per-"group" onto the 128 SBUF partitions
    (partition = 4*32 = pos_in_group x head).
  - Host pre-transposes q,k into [d, (pos,h)] layout and pre-casts to fp16,
    so the device needs no on-chip transposes and HBM traffic halves.
  - One 1.5 MB input DMA per 64-position chunk (q,k,v interleaved in one
    DRAM tensor) from the Sync HWDGE ring.
  - QK: per position j one col-tiled matmul (tile_position=(0,32j),
    K=128(d), M=32(k-heads), N=32(q-heads)); all 16 groups of a chunk land
    in one [128,512] PSUM bank -> a single exp() per chunk on ScalarE.
  - WV: per position a (32j,32j) sub-array matmul whose stationary operand
    is that position's [g,h] exp block read in place; V is stored
    [(pos,g), d] with a ones-column per group so the same matmul emits the
    softmax denominator in its last column.
  - Per-partition reciprocal (batched over 2 groups) + normalize while
    evacuating PSUM, alternating between ScalarE and VectorE.
  - Output halves drain early via the Scalar HWDGE ring so out-DMAs never
    head-of-line-block input prefetch on the Sync queue.
"""

import numpy as np

B, S, H, D = 2, 4096, 32, 128
NCORES = 8
POS = B * S                  # 8192 positions total
PPC = POS // NCORES          # 1024 positions per core
GP = 4                       # positions per group (4*32 heads = 128 partitions)
NG = 8                       # groups per chunk
CHUNK_POS = GP * NG          # 64 positions per chunk
NCHUNK = PPC // CHUNK_POS    # 16 chunks per core
VCOL = D + 1                 # v columns per group incl. ones column
QKW = NG * D                 # 2048 cols for qt / kt
VW = NG * VCOL               # 2064 cols for v
INW = 2 * QKW + VW           # combined input width per chunk

_SCALE = float(1.0 / np.sqrt(D))

_program = None  # cached compiled Bass program


def _build_program():
    import concourse.bacc as bacc
    import concourse.mybir as mybir
    from concourse.tile import TileContext

    fp32 = mybir.dt.float32
    fp16 = mybir.dt.float16

    nc = bacc.Bacc()
    qkv = nc.dram_tensor("qkv", [NCHUNK, 128, INW], fp16, kind="ExternalInput")
    out = nc.dram_tensor("out", [NCHUNK, 128, NG * D], fp16, kind="ExternalOutput")

    with TileContext(nc) as tc:
        with (
            tc.tile_pool(name="qkv_in", bufs=6) as in_pool,
            tc.tile_pool(name="o_out", bufs=4) as o_pool,
            tc.tile_pool(name="exp", bufs=3) as exp_pool,
            tc.tile_pool(name="small", bufs=8) as small_pool,
            tc.tile_pool(name="psl", bufs=3, space="PSUM") as psl_pool,
            tc.tile_pool(name="pso", bufs=5, space="PSUM") as pso_pool,
        ):
            for n in range(NCHUNK):
                in_t = in_pool.tile([128, INW], fp16, tag="qkv")
                nc.sync.dma_start(out=in_t, in_=qkv[n])
                qt_t = in_t[:, 0:QKW]
                kt_t = in_t[:, QKW:2 * QKW]
                vp_t = in_t[:, 2 * QKW:INW]
                out_t = o_pool.tile([128, NG * D], fp16, tag="out")

                # All 16 groups' logits^T into one [128, 512] PSUM bank.
                psum_l = psl_pool.tile([128, 32 * NG], fp32, tag="psl")
                for g in range(NG):
                    for j in range(GP):
                        c = slice(g * D + 32 * j, g * D + 32 * j + 32)
                        nc.tensor.matmul(
                            psum_l[32 * j:32 * j + 32, 32 * g:32 * g + 32],
                            kt_t[:, c],
                            qt_t[:, c],
                            start=True, stop=True,
                            tile_position=(0, 32 * j),
                        )
                exp_sb = exp_pool.tile([128, 32 * NG], fp16, tag="exp_sb")
                nc.scalar.activation(
                    exp_sb, psum_l, mybir.ActivationFunctionType.Exp,
                    scale=_SCALE,
                )

                for p2 in range(NG // 2):        # pairs of groups
                    psum_o = pso_pool.tile([128, 2 * VCOL], fp32, tag="pso")
                    for u in range(2):
                        g = 2 * p2 + u
                        for j in range(GP):
                            r = slice(32 * j, 32 * j + 32)
                            nc.tensor.matmul(
                                psum_o[r, u * VCOL:(u + 1) * VCOL],
                                exp_sb[r, 32 * g:32 * g + 32],
                                vp_t[r, g * VCOL:(g + 1) * VCOL],
                                start=True, stop=True,
                                tile_position=(32 * j, 32 * j),
                            )
                    recip = small_pool.tile([128, 2], fp32, tag="recip")
                    zcols = psum_o.rearrange("p (u c) -> p u c", c=VCOL)[:, :, D]
                    nc.vector.reciprocal(recip, zcols)
                    for u in range(2):
                        g = 2 * p2 + u
                        src = psum_o[:, u * VCOL:u * VCOL + D]
                        dst = out_t[:, g * D:(g + 1) * D]
                        if g % 2 == 0:
                            nc.scalar.activation(
                                dst, src, mybir.ActivationFunctionType.Copy,
                                scale=recip[:, u:u + 1],
                            )
                        else:
                            nc.vector.tensor_scalar_mul(dst, src, recip[:, u:u + 1])


                # Drain via the Scalar HWDGE ring so out-DMAs never
                # head-of-line-block input prefetch on the Sync queue.
                nc.scalar.dma_start(out=out[n], in_=out_t)

    nc.compile()
    return nc


def _host_pack(q, k, v):
    """Build per-core device input arrays from full fp32 inputs."""
    qf = np.ascontiguousarray(q, dtype=np.float32).reshape(POS, H, D)
    kf = np.ascontiguousarray(k, dtype=np.float32).reshape(POS, H, D)
    vf = np.ascontiguousarray(v, dtype=np.float32).reshape(POS, H, D)

    nchunk_tot = POS // CHUNK_POS
    # q,k: [chunk, group, i, h, d] -> [chunk, d, (group, i, h)]
    def to_qt(x):
        x = x.reshape(nchunk_tot, NG, GP, H, D)
        x = x.transpose(0, 4, 1, 2, 3)
        return x.reshape(nchunk_tot, D, NG * GP * H)

    # v: [chunk, group, i, gh, d] -> [chunk, (i,gh), (group, d|1)]
    vv = vf.reshape(nchunk_tot, NG, GP, H, D).transpose(0, 2, 3, 1, 4)
    vp_all = np.ones((nchunk_tot, GP, H, NG, VCOL), dtype=np.float32)
    vp_all[..., :D] = vv
    vp_all = vp_all.reshape(nchunk_tot, GP * H, NG * VCOL)

    comb = np.empty((nchunk_tot, 128, INW), dtype=np.float16)
    comb[:, :, 0:QKW] = to_qt(qf)
    comb[:, :, QKW:2 * QKW] = to_qt(kf)
    comb[:, :, 2 * QKW:INW] = vp_all

    in_maps = []
    for c in range(NCORES):
        sl = slice(c * NCHUNK, (c + 1) * NCHUNK)
        in_maps.append({"qkv": np.ascontiguousarray(comb[sl])})
    return in_maps


def _host_unpack(outs):
    """Per-core [NCHUNK, 128, NG*D] fp16 -> full [B, S, H*D] fp32."""
    full = np.concatenate(outs, axis=0).astype(np.float32)
    nchunk_tot = POS // CHUNK_POS
    full = full.reshape(nchunk_tot, GP, H, NG, D)   # [chunk, i, h, g, d]
    full = full.transpose(0, 3, 1, 2, 4)            # [chunk, g, i, h, d]
    return np.ascontiguousarray(full.reshape(B, S, H * D))


def kernel(q, k, v, _trace=False):
    global _program
    from concourse.bass_utils import run_bass_kernel_spmd

    if _program is None:
        _program = _build_program()

    in_maps = _host_pack(q, k, v)
    res = run_bass_kernel_spmd(_program, in_maps, list(range(NCORES)), trace=_trace)
    outs = [res.results[c]["out"] for c in range(NCORES)]
    result = _host_unpack(outs)
    if _trace:
        return result, res
    return result


# revision 13
# speedup vs baseline: 1.0801x; 1.0577x over previous
"""Trainium2 Bass kernel for per-position head-attention (nn_DariushFlashAttention2).

Math (per batch b, sequence position s):
    Q = q[b,s].reshape(H=32, D=128); K, V likewise
    logits = Q @ K.T / sqrt(D)          # [32, 32] attention over HEADS
    W = softmax(logits, axis=-1)
    out[b,s] = (W @ V).reshape(H*D)

Every one of the B*S = 8192 positions is independent, so we shard positions
across the 8 NeuronCores (1024 positions each) and run one SPMD program.

Device strategy (per core):
  - Positions are packed 4-per-"group" onto the 128 SBUF partitions
    (partition = 4*32 = pos_in_group x head).
  - Host pre-transposes q,k into [d, (pos,h)] layout and pre-casts to fp16,
    so the device needs no on-chip transposes and HBM traffic halves.
  - QK: per position j one col-tiled matmul (tile_position=(0,32j),
    K=128(d), M=32(k-heads), N=32(q-heads)) -> psum[32j:32j+32, 32t:+32]
    holds logits^T for that position only; no cross-position waste.
  - One exp() per 4 groups on ScalarE over the whole [128,128] psum tile.
  - WV: per position a (32j,32j) sub-array matmul whose stationary operand
    is that position's [g,h] exp block read in place; V is stored
    [(pos,g), d] with a ones-column per group so the same matmul emits the
    softmax denominator in its last column.
  - Per-partition reciprocal (batched over 2 groups) + normalize while
    evacuating PSUM, split 3:5 between ScalarE and VectorE.
  - Output halves drain early via the Scalar HWDGE ring so out-DMAs never
    head-of-line-block input prefetch on the Sync queue.
"""

import numpy as np

B, S, H, D = 2, 4096, 32, 128
NCORES = 8
POS = B * S                  # 8192 positions total
PPC = POS // NCORES          # 1024 positions per core
GP = 4                       # positions per group (4*32 heads = 128 partitions)
NG = 16                      # groups per chunk
CHUNK_POS = GP * NG          # 64 positions per chunk
NCHUNK = PPC // CHUNK_POS    # 16 chunks per core
VCOL = D + 1                 # v columns per group incl. ones column

_SCALE = float(1.0 / np.sqrt(D))

_program = None  # cached compiled Bass program


def _build_program():
    import concourse.bacc as bacc
    import concourse.mybir as mybir
    from concourse.tile import TileContext

    fp32 = mybir.dt.float32
    fp16 = mybir.dt.float16

    nc = bacc.Bacc()
    qt = nc.dram_tensor("qt", [NCHUNK, 128, NG * D], fp16, kind="ExternalInput")
    kt = nc.dram_tensor("kt", [NCHUNK, 128, NG * D], fp16, kind="ExternalInput")
    vp = nc.dram_tensor("vp", [NCHUNK, 128, NG * VCOL], fp16, kind="ExternalInput")
    out = nc.dram_tensor("out", [NCHUNK, 128, NG * D], fp16, kind="ExternalOutput")

    with TileContext(nc) as tc:
        with (
            tc.tile_pool(name="qk_in", bufs=4) as qk_pool,
            tc.tile_pool(name="v_in", bufs=4) as v_pool,
            tc.tile_pool(name="o_out", bufs=3) as o_pool,
            tc.tile_pool(name="exp", bufs=4) as exp_pool,
            tc.tile_pool(name="small", bufs=8) as small_pool,
            tc.tile_pool(name="psl", bufs=3, space="PSUM") as psl_pool,
            tc.tile_pool(name="pso", bufs=4, space="PSUM") as pso_pool,
        ):
            for n in range(NCHUNK):
                qt_t = qk_pool.tile([128, NG * D], fp16, tag="qt")
                kt_t = qk_pool.tile([128, NG * D], fp16, tag="kt")
                vp_t = v_pool.tile([128, NG * VCOL], fp16, tag="vp")
                nc.sync.dma_start(out=qt_t, in_=qt[n])
                nc.sync.dma_start(out=kt_t, in_=kt[n])
                nc.sync.dma_start(out=vp_t, in_=vp[n])
                out_t = o_pool.tile([128, NG * D], fp16, tag="out")

                for q4 in range(NG // 4):        # quad of groups
                    psum_l = psl_pool.tile([128, 128], fp32, tag="psl")
                    for t in range(4):           # group within quad
                        g = q4 * 4 + t
                        for j in range(GP):      # position within group
                            c = slice(g * D + 32 * j, g * D + 32 * j + 32)
                            nc.tensor.matmul(
                                psum_l[32 * j:32 * j + 32, 32 * t:32 * t + 32],
                                kt_t[:, c],
                                qt_t[:, c],
                                start=True, stop=True,
                                tile_position=(0, 32 * j),
                            )
                    exp_sb = exp_pool.tile([128, 128], fp16, tag="exp_sb")
                    nc.scalar.activation(
                        exp_sb, psum_l, mybir.ActivationFunctionType.Exp,
                        scale=_SCALE,
                    )
                    for p2 in range(2):          # pair of groups
                        psum_o = pso_pool.tile([128, 2 * VCOL], fp32, tag="pso")
                        for u in range(2):
                            g = q4 * 4 + p2 * 2 + u
                            t = p2 * 2 + u
                            for j in range(GP):
                                r = slice(32 * j, 32 * j + 32)
                                nc.tensor.matmul(
                                    psum_o[r, u * VCOL:(u + 1) * VCOL],
                                    exp_sb[r, 32 * t:32 * t + 32],
                                    vp_t[r, g * VCOL:(g + 1) * VCOL],
                                    start=True, stop=True,
                                    tile_position=(32 * j, 32 * j),
                                )
                        recip = small_pool.tile([128, 2], fp32, tag="recip")
                        zcols = psum_o.rearrange("p (u c) -> p u c", c=VCOL)[:, :, D]
                        nc.vector.reciprocal(recip, zcols)
                        for u in range(2):
                            g = q4 * 4 + p2 * 2 + u
                            src = psum_o[:, u * VCOL:u * VCOL + D]
                            dst = out_t[:, g * D:(g + 1) * D]
                            if g % 8 < 3:
                                nc.scalar.activation(
                                    dst, src, mybir.ActivationFunctionType.Copy,
                                    scale=recip[:, u:u + 1],
                                )
                            else:
                                nc.vector.tensor_scalar_mul(dst, src, recip[:, u:u + 1])

                    # Drain finished halves early from the Scalar HWDGE ring.
                    if q4 == 1:
                        nc.scalar.dma_start(
                            out=out[n, :, :NG * D // 2], in_=out_t[:, :NG * D // 2])
                    elif q4 == 3:
                        nc.scalar.dma_start(
                            out=out[n, :, NG * D // 2:], in_=out_t[:, NG * D // 2:])

    nc.compile()
    return nc


def _host_pack(q, k, v):
    """Build per-core device input arrays from full fp32 inputs."""
    qf = np.ascontiguousarray(q, dtype=np.float32).reshape(POS, H, D)
    kf = np.ascontiguousarray(k, dtype=np.float32).reshape(POS, H, D)
    vf = np.ascontiguousarray(v, dtype=np.float32).reshape(POS, H, D)

    nchunk_tot = POS // CHUNK_POS
    # q,k: [chunk, group, i, h, d] -> [chunk, d, (group, i, h)]
    def to_qt(x):
        x = x.reshape(nchunk_tot, NG, GP, H, D)
        x = x.transpose(0, 4, 1, 2, 3)
        return np.ascontiguousarray(x.reshape(nchunk_tot, D, NG * GP * H)).astype(np.float16)

    qt_all = to_qt(qf)
    kt_all = to_qt(kf)

    # v: [chunk, group, i, gh, d] -> [chunk, (i,gh), (group, d|1)]
    vv = vf.reshape(nchunk_tot, NG, GP, H, D).transpose(0, 2, 3, 1, 4)
    vp_all = np.ones((nchunk_tot, GP, H, NG, VCOL), dtype=np.float32)
    vp_all[..., :D] = vv
    vp_all = np.ascontiguousarray(
        vp_all.reshape(nchunk_tot, GP * H, NG * VCOL)
    ).astype(np.float16)

    in_maps = []
    for c in range(NCORES):
        sl = slice(c * NCHUNK, (c + 1) * NCHUNK)
        in_maps.append({
            "qt": np.ascontiguousarray(qt_all[sl]),
            "kt": np.ascontiguousarray(kt_all[sl]),
            "vp": np.ascontiguousarray(vp_all[sl]),
        })
    return in_maps


def _host_unpack(outs):
    """Per-core [NCHUNK, 128, NG*D] fp16 -> full [B, S, H*D] fp32."""
    full = np.concatenate(outs, axis=0).astype(np.float32)
    nchunk_tot = POS // CHUNK_POS
    full = full.reshape(nchunk_tot, GP, H, NG, D)   # [chunk, i, h, g, d]
    full = full.transpose(0, 3, 1, 2, 4)            # [chunk, g, i, h, d]
    return np.ascontiguousarray(full.reshape(B, S, H * D))


def kernel(q, k, v, _trace=False):
    global _program
    from concourse.bass_utils import run_bass_kernel_spmd

    if _program is None:
        _program = _build_program()

    in_maps = _host_pack(q, k, v)
    res = run_bass_kernel_spmd(_program, in_maps, list(range(NCORES)), trace=_trace)
    outs = [res.results[c]["out"] for c in range(NCORES)]
    result = _host_unpack(outs)
    if _trace:
        return result, res
    return result
